# revision 21
# baseline (speedup 1.0000x reference)
"""Trainium2 Bass kernel: 3-layer MLP (256->256->256->128) + action masking.

Sharding: pure data parallel. The batch dim (65536) is split across 8
NeuronCores (8192 rows each); the small MLP weights are replicated.

Layout: the host feeds each core FEATURE-MAJOR inputs (obs^T as bf16
[256, 8192], mask^T as int16 [128, 8192], weights packed/pre-transposed
as bf16) and transposes the feature-major f32 output [128, 8192] back to
batch-major afterward. With features on partitions end-to-end the kernel
needs ZERO PE transposes, and the bf16/int16 inputs halve the dominant
HBM read traffic (the memory roofline for this problem). The f32 logits
path (f32 PSUM accumulation, f32 output, exact FLOAT_MIN sentinels)
keeps norm rel-err ~4e-3 << the 2e-2 gate.

Input-dependent specialization (both variants are complete kernels):
  - "fast": used when no mask row is all-invalid AND b3 == 0 (checked on
    the host per call). Skips the all-invalid fixup machinery (colsum
    matmul, int16->bf16 mask convert, Relu(1-colsum) predicate, fixup
    copy) and the b3 ones-row matmul, all of which are exact no-ops for
    such inputs.
  - "safe": the general kernel with the exact all-invalid fixup
    (colsum = ones^T @ mask via PE; inv = Relu(1-colsum) as int32 on
    ACT; predicated 1.0 write to partition 0) and b3 folded in as a K=1
    ones-row matmul accumulated into the L3 PSUM group.

Per chunk (256-512 batch columns; small chunks at both ends shorten
pipeline fill/drain), software-pipelined with a 1-chunk skew per stage:
  A(c): DMA x^T / mask^T chunk (SP queue), FLOAT_MIN memset of the out
        tile (GPSIMD).
  B(c): L1 matmuls (moving = x^T bf16, 1 cycle/row) + relu/bias
        PSUM->SBUF (ACT, bf16 out).
  C(c): L2 matmuls; relu/bias m0 on DVE (tensor_scalar add+max), m1 on
        ACT to balance the two engines.
  D(c): L3 matmuls (logits finish IN PSUM - no separate bias copy);
        masking via copy_predicated straight from PSUM (DVE; masked
        entries stay exactly FLOAT_MIN); out-DMA issued from the DVE
        queue right after its producer, so it can never head-of-line
        block the SP input queue.
"""

import numpy as np

import concourse.bass as bass
import concourse.mybir as mybir
import concourse.tile as tile
from concourse import bacc
from concourse.bass_utils import run_bass_kernel_spmd

N_CORES = 8
B, S, F1, F2, A = 65536, 256, 256, 256, 128
B_CORE = B // N_CORES   # 8192
FLOAT_MIN = float(np.finfo(np.float32).min)

# chunk sizes: small at the ends (faster pipeline fill/drain), 512 steady
CHUNKS = [128, 128, 256] + [512] * 13 + [256] * 4
assert sum(CHUNKS) == B_CORE
NBMAX = 512

MM_DT = mybir.dt.bfloat16

# wrest image columns: [W2 (2*256) | W3 (2*128) | b3 row on partition 0]
WREST_W3_OFF = 512
WREST_B3_OFF = 768
WREST_COLS = 896


def _build(variant="fast", mm_dt=MM_DT):
    safe = variant == "safe"
    # Bacc (not plain Bass): its compile() pass splits multi-sem waits into
    # EventSemaphores - TRN2 instructions carry at most one wait.
    nc = bacc.Bacc(None, target_bir_lowering=False)
    f32 = mybir.dt.float32
    i32 = mybir.dt.int32
    i16 = mybir.dt.int16
    Relu = mybir.ActivationFunctionType.Relu

    obs = nc.dram_tensor("obs_state", [S, B_CORE], mm_dt, kind="ExternalInput")[:]
    msk = nc.dram_tensor("action_mask", [A, B_CORE], i16, kind="ExternalInput")[:]
    w1p = nc.dram_tensor("w1p", [128, 2 * F1], mm_dt, kind="ExternalInput")[:]
    wrest = nc.dram_tensor("wrest", [128, WREST_COLS], mm_dt, kind="ExternalInput")[:]
    bpack = nc.dram_tensor("bpack", [128, 4], f32, kind="ExternalInput")[:]
    out = nc.dram_tensor("out", [A, B_CORE], f32, kind="ExternalOutput")[:]

    # x^T rows (features) s = k*128 + p: k-block k on partition p, matching
    # the packed weights' layout so contraction dims align.
    obs_r = obs.rearrange("(k p) b -> p k b", p=128)

    with tile.TileContext(nc) as tc:
        with (
            tc.tile_pool(name="singles", bufs=1) as singles,
            tc.tile_pool(name="xp", bufs=6) as xp,
            tc.tile_pool(name="mp", bufs=6) as mp,
            tc.tile_pool(name="mfp", bufs=4) as mfp,
            tc.tile_pool(name="op", bufs=6) as op,
            tc.tile_pool(name="h1p", bufs=4) as h1p,
            tc.tile_pool(name="h2p", bufs=4) as h2p,
            tc.tile_pool(name="invp", bufs=3) as invp,
            # 8 PSUM banks total: fast 3+3+2; safe needs 2 for the colsum
            tc.tile_pool(name="ph1", bufs=2 if safe else 3, space="PSUM") as ph1,
            tc.tile_pool(name="ph2", bufs=2 if safe else 3, space="PSUM") as ph2,
            tc.tile_pool(name="plg", bufs=2, space="PSUM") as plg,
            tc.tile_pool(name="pcs", bufs=2 if safe else 1, space="PSUM") as pcs,
        ):
            # ---- one-time constants (no conversions needed) ----
            # Only w1p is emitted before the first x/mask chunk DMAs; the
            # rest of the constants follow them in the SP queue so L1(c0)
            # isn't delayed behind transfers it doesn't need.
            w1_sb = singles.tile([128, 2, F1], mm_dt)
            nc.sync.dma_start(w1_sb, w1p.rearrange("p (k f) -> p k f", k=2))
            wrest_sb = singles.tile([128, WREST_COLS], mm_dt)
            bp_sb = singles.tile([128, 4], f32)

            def load_rest_of_constants():
                nc.sync.dma_start(bp_sb, bpack)
                nc.sync.dma_start(wrest_sb, wrest)

            w2_sb = wrest_sb[:, 0:WREST_W3_OFF].rearrange("p (k f) -> p k f", k=2)
            w3_sb = wrest_sb[:, WREST_W3_OFF:WREST_B3_OFF].rearrange(
                "p (k f) -> p k f", k=2
            )
            b3_sb = wrest_sb[0:1, WREST_B3_OFF:WREST_B3_OFF + A]
            b1_sb = bp_sb[:, 0:2]
            b2_sb = bp_sb[:, 2:4]

            if safe:
                ones_row_f = singles.tile([1, NBMAX], f32)
                nc.vector.memset(ones_row_f, 1.0)
                ones_row_r = singles.tile([1, NBMAX], mm_dt)
                nc.vector.memset(ones_row_r, 1.0)
                onesA_r = singles.tile([128, 1], mm_dt)
                nc.vector.memset(onesA_r, 1.0)

            x_t, h1_t, h2_t, mask_t, maskf_t, out_t = {}, {}, {}, {}, {}, {}
            offs = np.concatenate([[0], np.cumsum(CHUNKS)]).tolist()

            def stage_a(c):
                nb = CHUNKS[c]
                sl = slice(offs[c], offs[c] + nb)
                x_t[c] = xp.tile([128, 2, nb], mm_dt, tag=f"x{nb}", name="x")
                nc.sync.dma_start(x_t[c], obs_r[:, :, sl])
                mask_t[c] = mp.tile([128, nb], i16, tag=f"mask{nb}", name="mask")
                nc.sync.dma_start(mask_t[c], msk[:, sl])
                out_t[c] = op.tile([128, nb], f32, tag=f"out{nb}", name="outt")
                nc.gpsimd.memset(out_t[c], FLOAT_MIN)
                if safe:
                    # int16 0/1 mask -> bf16 0.0/1.0 for the colsum matmul
                    # (CopyPredicated needs the int mask; matmul needs float)
                    maskf_t[c] = mfp.tile(
                        [128, nb], mm_dt, tag=f"mf{nb}", name="maskf"
                    )
                    nc.gpsimd.tensor_copy(maskf_t[c], mask_t[c])

            def stage_b(c):
                nb = CHUNKS[c]
                x_sb = x_t.pop(c)
                h1_t[c] = h1p.tile([128, 2, nb], mm_dt, tag=f"h1{nb}", name="h1")
                for m in range(2):
                    ps = ph1.tile([128, NBMAX], f32, tag="ph1")
                    for k in range(2):
                        nc.tensor.matmul(
                            ps[:, :nb],
                            w1_sb[:, k, m * 128 : (m + 1) * 128],
                            x_sb[:, k, :],
                            start=(k == 0),
                            stop=(k == 1),
                        )
                    nc.scalar.activation(
                        h1_t[c][:, m, :], ps[:, :nb], Relu, bias=b1_sb[:, m : m + 1]
                    )

            def stage_c(c):
                nb = CHUNKS[c]
                h1_sb = h1_t.pop(c)
                h2_t[c] = h2p.tile([128, 2, nb], mm_dt, tag=f"h2{nb}", name="h2")
                for m in range(2):
                    ps = ph2.tile([128, NBMAX], f32, tag="ph2")
                    for k in range(2):
                        nc.tensor.matmul(
                            ps[:, :nb],
                            w2_sb[:, k, m * 128 : (m + 1) * 128],
                            h1_sb[:, k, :],
                            start=(k == 0),
                            stop=(k == 1),
                        )
                    if m == 0:
                        nc.vector.tensor_scalar(
                            h2_t[c][:, m, :], ps[:, :nb],
                            b2_sb[:, m : m + 1], 0.0,
                            mybir.AluOpType.add, mybir.AluOpType.max,
                        )
                    else:
                        # balance ACT/DVE: second half of the relu on ACT
                        nc.scalar.activation(
                            h2_t[c][:, m, :], ps[:, :nb], Relu,
                            bias=b2_sb[:, m : m + 1],
                        )

            def stage_d(c):
                nb = CHUNKS[c]
                h2_sb = h2_t.pop(c)
                mask_sb = mask_t.pop(c)
                out_sb = out_t.pop(c)

                lg = plg.tile([128, NBMAX], f32, tag="plg")
                for k in range(2):
                    nc.tensor.matmul(
                        lg[:, :nb],
                        w3_sb[:, k, :],
                        h2_sb[:, k, :],
                        start=(k == 0),
                        stop=(k == 1) and not safe,
                    )
                if safe:
                    # b3: lg += b3^T (x) ones  (K=1 matmul closes the group)
                    nc.tensor.matmul(
                        lg[:, :nb], b3_sb, ones_row_r[:, :nb],
                        start=False, stop=True,
                    )
                    # all-invalid detection: colsum[b] = sum_a mask[a, b]
                    # (0/1 values, exact); inv = Relu(1 - colsum) as int32
                    # is exactly 1 iff colsum == 0.
                    maskf_sb = maskf_t.pop(c)
                    cs = pcs.tile([1, NBMAX], f32, tag="pcs")
                    nc.tensor.matmul(
                        cs[:, :nb], onesA_r, maskf_sb, start=True, stop=True
                    )
                    inv = invp.tile([1, nb], i32, tag=f"inv{nb}", name="inv")
                    nc.scalar.activation(
                        inv, cs[:, :nb], Relu, bias=1.0, scale=-1.0
                    )

                nc.vector.copy_predicated(out_sb, mask_sb, lg[:, :nb])
                if safe:
                    nc.vector.copy_predicated(
                        out_sb[0:1, :], inv, ones_row_f[:, :nb]
                    )
                # Output stream on the GPSIMD/SWDGE queue: its sem wait on
                # the masking ops can't block the SP input queue. The last
                # few chunks go out via ACT/HWDGE instead - at the drain
                # the input queues are idle, and this avoids serializing
                # the tail behind Pool's ~1us-per-DMA SWDGE generation.
                eng = nc.scalar if c >= len(CHUNKS) - 3 else nc.gpsimd
                eng.dma_start(out[:, offs[c] : offs[c] + nb], out_sb)

            n = len(CHUNKS)
            for i in range(n + 3):
                if i < n:
                    stage_a(i)
                if i == 0:
                    load_rest_of_constants()
                if 1 <= i < n + 1:
                    stage_b(i - 1)
                if 2 <= i < n + 2:
                    stage_c(i - 2)
                if 3 <= i:
                    stage_d(i - 3)

    return nc


_NC_CACHE = {}


def _get_nc(variant="fast", mm_dt=MM_DT):
    key = (variant, str(mm_dt))
    if key not in _NC_CACHE:
        nc = _build(variant, mm_dt)
        nc.finalize()
        _NC_CACHE[key] = nc
    return _NC_CACHE[key]


def _pack_weights(weights):
    """Host-side packing to the kernel's bf16 feature-major layouts."""
    import ml_dtypes

    bf16 = ml_dtypes.bfloat16
    W1, b1 = weights["W1"], weights["b1"]
    W2, b2 = weights["W2"], weights["b2"]
    W3, b3 = weights["W3"], weights["b3"]
    # (k p) f -> p (k f)
    w1p = np.ascontiguousarray(
        W1.reshape(2, 128, F1).transpose(1, 0, 2).reshape(128, 2 * F1)
    ).astype(bf16)
    wrest = np.zeros((128, WREST_COLS), dtype=bf16)
    wrest[:, 0:WREST_W3_OFF] = (
        W2.reshape(2, 128, F2).transpose(1, 0, 2).reshape(128, 2 * F2)
    ).astype(bf16)
    wrest[:, WREST_W3_OFF:WREST_B3_OFF] = (
        W3.reshape(2, 128, A).transpose(1, 0, 2).reshape(128, 2 * A)
    ).astype(bf16)
    wrest[0, WREST_B3_OFF:WREST_B3_OFF + A] = b3.astype(bf16)
    bpack = np.stack(
        [b1[:128], b1[128:], b2[:128], b2[128:]], axis=1
    ).astype(np.float32)
    return w1p, np.ascontiguousarray(wrest), np.ascontiguousarray(bpack)


def kernel(**inputs):
    import ml_dtypes

    bf16 = ml_dtypes.bfloat16
    obs = np.asarray(inputs["obs_state"], dtype=np.float32)
    msk = np.asarray(inputs["action_mask"])
    weights = {
        k: np.asarray(inputs[k], dtype=np.float32)
        for k in ("W1", "b1", "W2", "b2", "W3", "b3")
    }

    # Input-dependent dispatch: the fast kernel omits ops that are exact
    # no-ops when every row has a valid action and b3 == 0; the safe
    # kernel handles the general case.
    need_safe = bool((np.asarray(msk).sum(axis=1) == 0).any()) or bool(
        np.any(weights["b3"] != 0.0)
    )
    variant = "safe" if need_safe else "fast"

    # Feature-major host layout: obs^T (bf16) and mask^T (int16).
    obs_t = np.ascontiguousarray(obs.T.astype(bf16))           # [S, B]
    msk_t = np.ascontiguousarray(msk.T.astype(np.int16))       # [A, B]
    w1p, wrest, bpack = _pack_weights(weights)

    nc = _get_nc(variant)
    in_maps = []
    for i in range(N_CORES):
        sl = slice(i * B_CORE, (i + 1) * B_CORE)
        in_maps.append(
            {
                "obs_state": np.ascontiguousarray(obs_t[:, sl]),
                "action_mask": np.ascontiguousarray(msk_t[:, sl]),
                "w1p": w1p,
                "wrest": wrest,
                "bpack": bpack,
            }
        )
    res = run_bass_kernel_spmd(nc, in_maps, core_ids=list(range(N_CORES)))
    out_t = np.concatenate([r["out"] for r in res.results], axis=1)  # [A, B]
    return np.ascontiguousarray(out_t.T)


if __name__ == "__main__":
    nc = _get_nc()
    print("build OK")
